# revision 65
# baseline (speedup 1.0000x reference)
"""Bahdanau additive attention on 8 Trainium2 NeuronCores.

Reference computation (per full batch B=32, T=2048, H=U=1024):
    q = dec @ W1 + b1                         [B, 1, U]
    k = enc @ W2 + b2                         [B, T, U]
    score = tanh(q + k) @ V + bv              [B, T, 1]
    attn = softmax(score, axis=T)             [B, T, 1]
    context = sum(attn * enc, axis=T)         [B, H]
    returns (context, attn)

Sharding: data-parallel over batch, 4 batches per core; weights replicated.
enc/W1/W2/V are converted to bf16 on the host (total error ~3e-3 rel vs a
2e-2 gate); all matmuls run bf16 with f32 PSUM accumulation.

Per-core dataflow:
    - kT per 512-t block: xbar DMA-transpose straight from DRAM gives
      encT [h,t]; W2 chunks (stationary) x encT -> PSUM [u,t].
    - tanh fused with the +q per-partition bias on ScalarE out of PSUM.
    - score via V.T (stationary) x tanh tile -> PSUM [1,t], pipelined one
      uc-chain behind the k-projection so the PE never waits on ACT.
    - softmax shift-free: e=exp(s); attn = e / sum(e)  (bv cancels).
    - scores transposed [1,T]->[T,1] chunks on TensorE; eT=exp -> lhsT for
      the context matmul against a natural-layout enc copy.

Scheduling notes (the perf-critical part):
    - Tile serializes DMA_TRANSPOSE against in-flight plain-copy DMAs
      (xbar-mode transition guard) and its scheduler interleaves them
      freely, which costs a drain at every mode switch. The transpose
      stream is therefore pinned into a total order with plain copies
      confined to designated inter-burst gaps (add_dep_helper edges).
    - Startup weight loads ride three parallel DMA paths (SWDGE + both
      HWDGE rings); score-phase copies run on DVE to keep the ACT queue
      a pure tanh stream.
"""

import numpy as np

P = 128
B = 32
N_CORES = 8
BLOC = B // N_CORES  # 4 batches per core
T = 2048
H = 1024
U = 1024
HC = H // P  # 8 h-chunks
UC = U // P  # 8 u-chunks
TT = T // P  # 16 t-tiles per batch
TB = 4       # t-tiles per t-block
NTB = TT // TB  # 4 t-blocks
TN = TB * P  # 512

_CACHE = {}


def _build():
    import concourse.mybir as mybir
    import concourse.tile as tile
    from concourse import bacc
    from concourse.masks import make_identity
    from concourse.tile import add_dep_helper

    f32 = mybir.dt.float32
    bf16 = mybir.dt.bfloat16
    AF = mybir.ActivationFunctionType

    nc = bacc.Bacc(None, target_bir_lowering=False)

    # Host pre-arranges small tensors into on-chip layouts:
    #   dect[p, hc, b] = dec[b, hc*128+p] (bf16)
    #   biasu[p, uc]   = b1[uc*128+p] + b2[uc*128+p] (f32)
    #   vt[p, uc]      = V[uc*128+p, 0] (bf16)
    enc = nc.declare_dram_parameter("enc", [BLOC, T, H], bf16, isOutput=False)
    w1 = nc.declare_dram_parameter("w1", [H, U], bf16, isOutput=False)
    w2 = nc.declare_dram_parameter("w2", [H, U], bf16, isOutput=False)
    dect = nc.declare_dram_parameter("dect", [P, HC, BLOC], bf16, isOutput=False)
    biasp = nc.declare_dram_parameter("biasu", [P, UC], f32, isOutput=False)
    vtp = nc.declare_dram_parameter("vt", [P, UC], bf16, isOutput=False)
    octx = nc.declare_dram_parameter("out_ctx", [BLOC, H], f32, isOutput=True)
    oattn = nc.declare_dram_parameter("out_attn", [BLOC, T], f32, isOutput=True)

    from contextlib import ExitStack

    with tile.TileContext(nc) as tc, ExitStack() as ctx:
        consts = ctx.enter_context(tc.tile_pool(name="consts", bufs=1))
        encp = ctx.enter_context(tc.tile_pool(name="encp", bufs=2))
        encTp = ctx.enter_context(tc.tile_pool(name="encTp", bufs=3))
        thp = ctx.enter_context(tc.tile_pool(name="thp", bufs=4))
        sp = ctx.enter_context(tc.tile_pool(name="sp", bufs=2))
        psk = ctx.enter_context(tc.tile_pool(name="psk", bufs=3, space="PSUM"))
        pss = ctx.enter_context(tc.tile_pool(name="pss", bufs=2, space="PSUM"))
        psm = ctx.enter_context(tc.tile_pool(name="psm", bufs=1, space="PSUM"))
        pst = ctx.enter_context(tc.tile_pool(name="pst", bufs=1, space="PSUM"))

        # ---- constants ----
        # ALL plain-copy DMAs ride gpsimd/SWDGE; the sync HWDGE ring carries
        # ONLY the xbar transposes. Rationale: Tile serializes DMA_TRANSPOSE
        # against in-flight HWDGE plain copies (xbar-mode transition guard),
        # but SWDGE copies don't participate — keeping the two streams on
        # disjoint paths lets them pipeline freely.
        const_lds = []
        decT = consts.tile([P, HC, BLOC], bf16)
        const_lds.append(nc.gpsimd.dma_start(out=decT[:], in_=dect.ap()))
        biasu = consts.tile([P, UC], f32)
        const_lds.append(nc.gpsimd.dma_start(out=biasu[:], in_=biasp.ap()))
        vsb = consts.tile([P, UC], bf16)
        const_lds.append(nc.gpsimd.dma_start(out=vsb[:], in_=vtp.ap()))
        # w1 leads the sync HWDGE ring (transposes follow it there).
        w1sb = consts.tile([P, HC, U], bf16)
        const_lds.append(nc.sync.dma_start(
            out=w1sb[:], in_=w1.ap().rearrange("(hc p) u -> p hc u", p=P)))
        # w2 on the ACT HWDGE ring, in parallel with w1 on the SWDGE queue.
        w2sb = consts.tile([P, HC, U], bf16)
        const_lds.append(nc.scalar.dma_start(
            out=w2sb[:], in_=w2.ap().rearrange("(hc p) u -> p hc u", p=P)))
        ident = consts.tile([P, P], f32)
        make_identity(nc, ident[:])

        # qT[u, b] = (dec @ W1)[b, u] + b1[u] + b2[u]
        qT = consts.tile([P, UC, BLOC], f32)
        for uc in range(UC):
            qpsum = psk.tile([P, TN], f32, tag="k", name=f"qpsum{uc}")
            for hc in range(HC):
                nc.tensor.matmul(
                    qpsum[:, :BLOC],
                    lhsT=w1sb[:, hc, uc * P:(uc + 1) * P],
                    rhs=decT[:, hc, :],
                    start=(hc == 0),
                    stop=(hc == HC - 1),
                )
            # On DVE (not ACT) so it can't sit behind a DMA emission on the
            # ACT queue while the PE waits on the qpsum slot.
            nc.vector.tensor_scalar_add(
                out=qT[:, uc, :], in0=qpsum[:, :BLOC], scalar1=biasu[:, uc:uc + 1]
            )

        # ---- main loop over local batches ----
        # Every transpose<->copy mode transition on the DMA path costs a
        # drain (Tile's xbar-transition guard), and the scheduler interleaves
        # them freely. Pin a global order with explicit ordering edges:
        # copies only in designated gaps between transpose bursts.
        tr_groups = {}   # (b, tb) -> [transpose insts]
        copy_slots = {}  # (b, tb) -> [plain-copy insts to place after group]
        encb_tiles = []
        for b in range(BLOC):
            encb = encp.tile([P, TT, H], bf16, tag="enc", name=f"encb{b}")
            encb_tiles.append(encb)
            enc_src = enc.ap()[b].rearrange("(tt p) h -> p tt h", p=P)
            ssb = sp.tile([1, T], f32, tag="s", name=f"s{b}")
            if b == BLOC - 1:
                eTl = sp.tile([P, TT], bf16, tag="eT", name=f"eTl{b}")
                pctxl = psm.tile([1, H], f32, tag="pctx", name=f"pctxl{b}")
            # Natural-layout copy (context matmul input), scheduled into the
            # gap after this batch's tb1 transpose burst.
            ld = nc.gpsimd.dma_start(out=encb[:], in_=enc_src)
            copy_slots.setdefault((b, 1), []).append(ld)
            for tb in range(NTB):
                encT = encTp.tile([P, HC, TB, P], bf16, tag="encT", name=f"encT{b}_{tb}")
                grp = []
                for tt in range(TB):
                    # xbar transpose straight from DRAM: [128t, 1024h] -> [128h, 8hc, 128t]
                    grp.append(nc.sync.dma_start_transpose(
                        encT[:, :, tt, :],
                        enc.ap()[b, (tb * TB + tt) * P:(tb * TB + tt + 1) * P, :],
                    ))
                tr_groups[(b, tb)] = grp
                pscore = pss.tile([1, TN], f32, tag="ps", name=f"pscore{b}_{tb}")
                # k-projection chains, with the V-score matmul pipelined one
                # uc-chain behind so the PE never waits on the tanh drain.
                ths = []
                for uc in range(UC):
                    pk = psk.tile([P, TN], f32, tag="k", name=f"pk{b}_{tb}_{uc}")
                    for hc in range(HC):
                        nc.tensor.matmul(
                            pk[:],
                            lhsT=w2sb[:, hc, uc * P:(uc + 1) * P],
                            rhs=encT[:, hc, :, :],
                            start=(hc == 0),
                            stop=(hc == HC - 1),
                        )
                    th = thp.tile([P, TN], bf16, tag="th", name=f"th{b}_{tb}_{uc}")
                    nc.scalar.activation(
                        out=th[:], in_=pk[:], func=AF.Tanh,
                        bias=qT[:, uc, b:b + 1], scale=1.0,
                    )
                    ths.append(th)
                    if uc >= 1:
                        nc.tensor.matmul(
                            pscore[:],
                            lhsT=vsb[:, uc - 1:uc],
                            rhs=ths[uc - 1][:],
                            start=(uc - 1 == 0),
                            stop=False,
                        )
                nc.tensor.matmul(
                    pscore[:],
                    lhsT=vsb[:, UC - 1:UC],
                    rhs=ths[UC - 1][:],
                    start=False,
                    stop=True,
                )
                # DVE, not ACT: keeps the tanh stream uninterrupted and the
                # score->transpose chain off the busy ACT queue.
                nc.vector.tensor_copy(out=ssb[:, tb * TN:(tb + 1) * TN], in_=pscore[:])
                if b == BLOC - 1:
                    # Last batch: transpose+exp this block's scores right
                    # away — there is no next batch to hide the clump under,
                    # so shrinking the serial tail matters here. The context
                    # matmuls for the block follow for the same reason.
                    pTi = pst.tile([P, TB], f32, tag="pT", name=f"pTi{b}_{tb}")
                    for o4 in range(TB):
                        o = tb * TB + o4
                        nc.tensor.transpose(
                            pTi[:, o4:o4 + 1], ssb[:, o * P:(o + 1) * P], ident[:1, :1])
                    nc.scalar.activation(
                        out=eTl[:, tb * TB:(tb + 1) * TB], in_=pTi[:], func=AF.Exp)
                    for o4 in range(TB):
                        o = tb * TB + o4
                        for h2 in range(2):
                            nc.tensor.matmul(
                                pctxl[:, h2 * 512:(h2 + 1) * 512],
                                lhsT=eTl[:, o:o + 1],
                                rhs=encb[:, o, h2 * 512:(h2 + 1) * 512],
                                start=(o == 0),
                                stop=(o == TT - 1),
                            )

            # softmax (shift-free: scores are O(1) here)
            esb = sp.tile([1, T], f32, tag="e", name=f"e{b}")
            nc.scalar.activation(out=esb[:], in_=ssb[:], func=AF.Exp)
            ssum = sp.tile([1, 1], f32, tag="ssum", name=f"ssum{b}")
            nc.vector.reduce_sum(out=ssum[:], in_=esb[:], axis=mybir.AxisListType.X)
            rinv = sp.tile([1, 1], f32, tag="rinv", name=f"rinv{b}")
            nc.vector.reciprocal(out=rinv[:], in_=ssum[:])
            attn_sb = sp.tile([1, T], f32, tag="attn", name=f"attn{b}")
            nc.vector.tensor_scalar_mul(out=attn_sb[:], in0=esb[:], scalar1=rinv[:])
            od = nc.gpsimd.dma_start(out=oattn.ap()[b:b + 1, :], in_=attn_sb[:])
            # outputs of batch b land in the gap after batch b+1's tb2 burst
            # (data is long ready by then); last batch goes after the end.
            copy_slots.setdefault((min(b + 1, BLOC - 1), 2 if b < BLOC - 1 else 3), []).append(od)

            # scores -> [t, 1] chunks on TensorE, then eT = exp(sT) as bf16
            # (already produced per t-block for the last batch)
            if b == BLOC - 1:
                eT = eTl
            else:
                pT = pst.tile([P, TT], f32, tag="pT", name=f"pT{b}")
                for o in range(TT):
                    nc.tensor.transpose(pT[:, o:o + 1], ssb[:, o * P:(o + 1) * P], ident[:1, :1])
                eT = sp.tile([P, TT], bf16, tag="eT", name=f"eT{b}")
                nc.scalar.activation(out=eT[:], in_=pT[:], func=AF.Exp)

            # context = (eT / Z).T @ enc (last batch accumulated per-block)
            if b == BLOC - 1:
                pctx = pctxl
            else:
                pctx = psm.tile([1, H], f32, tag="pctx", name=f"pctx{b}")
                for o in range(TT):
                    for h2 in range(2):
                        nc.tensor.matmul(
                            pctx[:, h2 * 512:(h2 + 1) * 512],
                            lhsT=eT[:, o:o + 1],
                            rhs=encb[:, o, h2 * 512:(h2 + 1) * 512],
                            start=(o == 0),
                            stop=(o == TT - 1),
                        )
            ctx_sb = sp.tile([1, H], f32, tag="ctx", name=f"ctx{b}")
            nc.vector.tensor_scalar_mul(out=ctx_sb[:], in0=pctx[:], scalar1=rinv[:])
            od = nc.gpsimd.dma_start(out=octx.ap()[b:b + 1, :], in_=ctx_sb[:])
            copy_slots.setdefault((min(b + 1, BLOC - 1), 2 if b < BLOC - 1 else 3), []).append(od)

        # Pin the global DMA order: [consts..w2] -> burst(0,0) -> ... with
        # plain copies only in their designated inter-burst gaps.
        # add_dep_helper(dependent, dependency).
        order = [(b, tb) for b in range(BLOC) for tb in range(NTB)]
        # Head: const copies (w1 SWDGE || w2 ACT-ring) first, then the
        # transpose stream begins.
        for cp in const_lds:
            add_dep_helper(
                tr_groups[order[0]][0].ins, cp.ins, sync=False,
                reason="first transpose burst after const copies",
            )
        prev_tr = None
        for key in order:
            for tr in tr_groups[key]:
                if prev_tr is not None:
                    add_dep_helper(
                        tr.ins, prev_tr.ins, sync=False,
                        reason="transpose stream total order",
                    )
                prev_tr = tr
        for gi, key in enumerate(order):
            grp = tr_groups[key]
            nxt = tr_groups[order[gi + 1]][0] if gi + 1 < len(order) else None
            for cp in copy_slots.get(key, []):
                add_dep_helper(
                    cp.ins, grp[-1].ins, sync=False,
                    reason="plain copy after transpose burst",
                )
                if nxt is not None:
                    add_dep_helper(
                        nxt.ins, cp.ins, sync=False,
                        reason="next transpose burst after gap copies",
                    )

    nc.finalize()
    return nc


def kernel(dec_hidden, all_enc_hiddens, W1, b1, W2, b2, V, bv):
    from concourse.bass_utils import run_bass_kernel_spmd

    if "nc" not in _CACHE:
        _CACHE["nc"] = _build()
    nc = _CACHE["nc"]

    import ml_dtypes
    bf = ml_dtypes.bfloat16
    dec_hidden = np.asarray(dec_hidden, dtype=np.float32)
    all_enc_hiddens = np.ascontiguousarray(np.asarray(all_enc_hiddens, dtype=np.float32).astype(bf))
    W1 = np.ascontiguousarray(np.asarray(W1, dtype=np.float32).astype(bf))
    W2 = np.ascontiguousarray(np.asarray(W2, dtype=np.float32).astype(bf))
    b1 = np.asarray(b1, dtype=np.float32)
    b2 = np.asarray(b2, dtype=np.float32)
    V = np.asarray(V, dtype=np.float32)

    # Host-side layout prep for the small tensors (see _build docstring).
    biasu = np.ascontiguousarray((b1 + b2).reshape(UC, P).T.astype(np.float32))
    vt = np.ascontiguousarray(V[:, 0].reshape(UC, P).T.astype(bf))

    in_maps = []
    for i in range(N_CORES):
        sl = slice(i * BLOC, (i + 1) * BLOC)
        dec_core = dec_hidden[sl]  # [BLOC, H]
        dect = np.ascontiguousarray(
            dec_core.T.reshape(HC, P, BLOC).transpose(1, 0, 2).astype(bf)
        )
        in_maps.append({
            "enc": np.ascontiguousarray(all_enc_hiddens[sl]),
            "w1": W1, "w2": W2, "dect": dect, "biasu": biasu, "vt": vt,
        })

    res = run_bass_kernel_spmd(nc, in_maps, core_ids=list(range(N_CORES)))
    context = np.concatenate([res.results[i]["out_ctx"] for i in range(N_CORES)], axis=0)
    attn = np.concatenate([res.results[i]["out_attn"] for i in range(N_CORES)], axis=0)
    return context.astype(np.float32), attn.reshape(B, T, 1).astype(np.float32)


# revision 66
# speedup vs baseline: 1.0195x; 1.0195x over previous
"""Bahdanau additive attention on 8 Trainium2 NeuronCores.

Reference computation (per full batch B=32, T=2048, H=U=1024):
    q = dec @ W1 + b1                         [B, 1, U]
    k = enc @ W2 + b2                         [B, T, U]
    score = tanh(q + k) @ V + bv              [B, T, 1]
    attn = softmax(score, axis=T)             [B, T, 1]
    context = sum(attn * enc, axis=T)         [B, H]
    returns (context, attn)

Sharding: data-parallel over batch, 4 batches per core; weights replicated.
enc/W1/W2/V are converted to bf16 on the host (total error ~3e-3 rel vs a
2e-2 gate); all matmuls run bf16 with f32 PSUM accumulation.

Per-core dataflow:
    - kT per 512-t block: xbar DMA-transpose straight from DRAM gives
      encT [h,t]; W2 chunks (stationary) x encT -> PSUM [u,t].
    - tanh fused with the +q per-partition bias on ScalarE out of PSUM.
    - score via V.T (stationary) x tanh tile -> PSUM [1,t], pipelined one
      uc-chain behind the k-projection so the PE never waits on ACT.
    - softmax shift-free: e=exp(s); attn = e / sum(e)  (bv cancels).
    - scores transposed [1,T]->[T,1] chunks on TensorE; eT=exp -> lhsT for
      the context matmul against a natural-layout enc copy.

Scheduling notes (the perf-critical part):
    - Tile serializes DMA_TRANSPOSE against in-flight plain-copy DMAs
      (xbar-mode transition guard) and its scheduler interleaves them
      freely, which costs a drain at every mode switch. The transpose
      stream is therefore pinned into a total order with plain copies
      confined to designated inter-burst gaps (add_dep_helper edges).
    - Startup weight loads ride three parallel DMA paths (SWDGE + both
      HWDGE rings); score-phase copies run on DVE to keep the ACT queue
      a pure tanh stream.
"""

import numpy as np

P = 128
B = 32
N_CORES = 8
BLOC = B // N_CORES  # 4 batches per core
T = 2048
H = 1024
U = 1024
HC = H // P  # 8 h-chunks
UC = U // P  # 8 u-chunks
TT = T // P  # 16 t-tiles per batch
TB = 4       # t-tiles per t-block
NTB = TT // TB  # 4 t-blocks
TN = TB * P  # 512

_CACHE = {}


def _build():
    import concourse.mybir as mybir
    import concourse.tile as tile
    from concourse import bacc
    from concourse.masks import make_identity
    from concourse.tile import add_dep_helper

    f32 = mybir.dt.float32
    bf16 = mybir.dt.bfloat16
    AF = mybir.ActivationFunctionType

    nc = bacc.Bacc(None, target_bir_lowering=False)

    # Host pre-arranges small tensors into on-chip layouts:
    #   dect[p, hc, b] = dec[b, hc*128+p] (bf16)
    #   biasu[p, uc]   = b1[uc*128+p] + b2[uc*128+p] (f32)
    #   vt[p, uc]      = V[uc*128+p, 0] (bf16)
    enc = nc.declare_dram_parameter("enc", [BLOC, T, H], bf16, isOutput=False)
    w1 = nc.declare_dram_parameter("w1", [H, U], bf16, isOutput=False)
    w2 = nc.declare_dram_parameter("w2", [H, U], bf16, isOutput=False)
    dect = nc.declare_dram_parameter("dect", [P, HC, BLOC], bf16, isOutput=False)
    biasp = nc.declare_dram_parameter("biasu", [P, UC], f32, isOutput=False)
    vtp = nc.declare_dram_parameter("vt", [P, UC], bf16, isOutput=False)
    octx = nc.declare_dram_parameter("out_ctx", [BLOC, H], f32, isOutput=True)
    oattn = nc.declare_dram_parameter("out_attn", [BLOC, T], f32, isOutput=True)

    from contextlib import ExitStack

    with tile.TileContext(nc) as tc, ExitStack() as ctx:
        consts = ctx.enter_context(tc.tile_pool(name="consts", bufs=1))
        encp = ctx.enter_context(tc.tile_pool(name="encp", bufs=2))
        encTp = ctx.enter_context(tc.tile_pool(name="encTp", bufs=3))
        thp = ctx.enter_context(tc.tile_pool(name="thp", bufs=4))
        sp = ctx.enter_context(tc.tile_pool(name="sp", bufs=2))
        psk = ctx.enter_context(tc.tile_pool(name="psk", bufs=3, space="PSUM"))
        pss = ctx.enter_context(tc.tile_pool(name="pss", bufs=2, space="PSUM"))
        psm = ctx.enter_context(tc.tile_pool(name="psm", bufs=1, space="PSUM"))
        pst = ctx.enter_context(tc.tile_pool(name="pst", bufs=1, space="PSUM"))

        # ---- constants ----
        # ALL plain-copy DMAs ride gpsimd/SWDGE; the sync HWDGE ring carries
        # ONLY the xbar transposes. Rationale: Tile serializes DMA_TRANSPOSE
        # against in-flight HWDGE plain copies (xbar-mode transition guard),
        # but SWDGE copies don't participate — keeping the two streams on
        # disjoint paths lets them pipeline freely.
        const_lds = []
        decT = consts.tile([P, HC, BLOC], bf16)
        const_lds.append(nc.gpsimd.dma_start(out=decT[:], in_=dect.ap()))
        biasu = consts.tile([P, UC], f32)
        const_lds.append(nc.gpsimd.dma_start(out=biasu[:], in_=biasp.ap()))
        vsb = consts.tile([P, UC], bf16)
        const_lds.append(nc.gpsimd.dma_start(out=vsb[:], in_=vtp.ap()))
        # w1 leads the sync HWDGE ring (transposes follow it there).
        w1sb = consts.tile([P, HC, U], bf16)
        const_lds.append(nc.sync.dma_start(
            out=w1sb[:], in_=w1.ap().rearrange("(hc p) u -> p hc u", p=P)))
        # w2 on the ACT HWDGE ring, in parallel with w1 on the SWDGE queue.
        w2sb = consts.tile([P, HC, U], bf16)
        const_lds.append(nc.scalar.dma_start(
            out=w2sb[:], in_=w2.ap().rearrange("(hc p) u -> p hc u", p=P)))
        ident = consts.tile([P, P], f32)
        make_identity(nc, ident[:])

        # qT[u, b] = (dec @ W1)[b, u] + b1[u] + b2[u]
        qT = consts.tile([P, UC, BLOC], f32)
        for uc in range(UC):
            qpsum = psk.tile([P, TN], f32, tag="k", name=f"qpsum{uc}")
            for hc in range(HC):
                nc.tensor.matmul(
                    qpsum[:, :BLOC],
                    lhsT=w1sb[:, hc, uc * P:(uc + 1) * P],
                    rhs=decT[:, hc, :],
                    start=(hc == 0),
                    stop=(hc == HC - 1),
                )
            # On DVE (not ACT) so it can't sit behind a DMA emission on the
            # ACT queue while the PE waits on the qpsum slot.
            nc.vector.tensor_scalar_add(
                out=qT[:, uc, :], in0=qpsum[:, :BLOC], scalar1=biasu[:, uc:uc + 1]
            )

        # ---- main loop over local batches ----
        # Every transpose<->copy mode transition on the DMA path costs a
        # drain (Tile's xbar-transition guard), and the scheduler interleaves
        # them freely. Pin a global order with explicit ordering edges:
        # copies only in designated gaps between transpose bursts.
        tr_groups = {}   # (b, tb) -> [transpose insts]
        copy_slots = {}  # (b, tb) -> [plain-copy insts to place after group]
        encb_tiles = []
        for b in range(BLOC):
            encb = encp.tile([P, TT, H], bf16, tag="enc", name=f"encb{b}")
            encb_tiles.append(encb)
            enc_src = enc.ap()[b].rearrange("(tt p) h -> p tt h", p=P)
            ssb = sp.tile([1, T], f32, tag="s", name=f"s{b}")
            if b == BLOC - 1:
                eTl = sp.tile([P, TT], bf16, tag="eT", name=f"eTl{b}")
            # Natural-layout copy (context matmul input), scheduled into the
            # gap after this batch's tb1 transpose burst.
            ld = nc.gpsimd.dma_start(out=encb[:], in_=enc_src)
            copy_slots.setdefault((b, 1), []).append(ld)
            for tb in range(NTB):
                encT = encTp.tile([P, HC, TB, P], bf16, tag="encT", name=f"encT{b}_{tb}")
                grp = []
                for tt in range(TB):
                    # xbar transpose straight from DRAM: [128t, 1024h] -> [128h, 8hc, 128t]
                    grp.append(nc.sync.dma_start_transpose(
                        encT[:, :, tt, :],
                        enc.ap()[b, (tb * TB + tt) * P:(tb * TB + tt + 1) * P, :],
                    ))
                tr_groups[(b, tb)] = grp
                pscore = pss.tile([1, TN], f32, tag="ps", name=f"pscore{b}_{tb}")
                # k-projection chains, with the V-score matmul pipelined one
                # uc-chain behind so the PE never waits on the tanh drain.
                ths = []
                for uc in range(UC):
                    pk = psk.tile([P, TN], f32, tag="k", name=f"pk{b}_{tb}_{uc}")
                    for hc in range(HC):
                        nc.tensor.matmul(
                            pk[:],
                            lhsT=w2sb[:, hc, uc * P:(uc + 1) * P],
                            rhs=encT[:, hc, :, :],
                            start=(hc == 0),
                            stop=(hc == HC - 1),
                        )
                    th = thp.tile([P, TN], bf16, tag="th", name=f"th{b}_{tb}_{uc}")
                    nc.scalar.activation(
                        out=th[:], in_=pk[:], func=AF.Tanh,
                        bias=qT[:, uc, b:b + 1], scale=1.0,
                    )
                    ths.append(th)
                    if uc >= 1:
                        nc.tensor.matmul(
                            pscore[:],
                            lhsT=vsb[:, uc - 1:uc],
                            rhs=ths[uc - 1][:],
                            start=(uc - 1 == 0),
                            stop=False,
                        )
                nc.tensor.matmul(
                    pscore[:],
                    lhsT=vsb[:, UC - 1:UC],
                    rhs=ths[UC - 1][:],
                    start=False,
                    stop=True,
                )
                # DVE, not ACT: keeps the tanh stream uninterrupted and the
                # score->transpose chain off the busy ACT queue.
                nc.vector.tensor_copy(out=ssb[:, tb * TN:(tb + 1) * TN], in_=pscore[:])
                if b == BLOC - 1:
                    # Last batch: transpose+exp this block's scores right
                    # away — there is no next batch to hide the clump under,
                    # so shrinking the serial tail matters here.
                    pTi = pst.tile([P, TB], f32, tag="pT", name=f"pTi{b}_{tb}")
                    for o4 in range(TB):
                        o = tb * TB + o4
                        nc.tensor.transpose(
                            pTi[:, o4:o4 + 1], ssb[:, o * P:(o + 1) * P], ident[:1, :1])
                    nc.scalar.activation(
                        out=eTl[:, tb * TB:(tb + 1) * TB], in_=pTi[:], func=AF.Exp)

            # softmax (shift-free: scores are O(1) here)
            esb = sp.tile([1, T], f32, tag="e", name=f"e{b}")
            nc.scalar.activation(out=esb[:], in_=ssb[:], func=AF.Exp)
            ssum = sp.tile([1, 1], f32, tag="ssum", name=f"ssum{b}")
            nc.vector.reduce_sum(out=ssum[:], in_=esb[:], axis=mybir.AxisListType.X)
            rinv = sp.tile([1, 1], f32, tag="rinv", name=f"rinv{b}")
            nc.vector.reciprocal(out=rinv[:], in_=ssum[:])
            attn_sb = sp.tile([1, T], f32, tag="attn", name=f"attn{b}")
            nc.vector.tensor_scalar_mul(out=attn_sb[:], in0=esb[:], scalar1=rinv[:])
            od = nc.gpsimd.dma_start(out=oattn.ap()[b:b + 1, :], in_=attn_sb[:])
            # outputs of batch b land in the gap after batch b+1's tb2 burst
            # (data is long ready by then); last batch goes after the end.
            copy_slots.setdefault((min(b + 1, BLOC - 1), 2 if b < BLOC - 1 else 3), []).append(od)

            # scores -> [t, 1] chunks on TensorE, then eT = exp(sT) as bf16
            # (already produced per t-block for the last batch)
            if b == BLOC - 1:
                eT = eTl
            else:
                pT = pst.tile([P, TT], f32, tag="pT", name=f"pT{b}")
                for o in range(TT):
                    nc.tensor.transpose(pT[:, o:o + 1], ssb[:, o * P:(o + 1) * P], ident[:1, :1])
                eT = sp.tile([P, TT], bf16, tag="eT", name=f"eT{b}")
                nc.scalar.activation(out=eT[:], in_=pT[:], func=AF.Exp)

            # context = (eT / Z).T @ enc
            pctx = psm.tile([1, H], f32, tag="pctx", name=f"pctx{b}")
            for o in range(TT):
                for h2 in range(2):
                    nc.tensor.matmul(
                        pctx[:, h2 * 512:(h2 + 1) * 512],
                        lhsT=eT[:, o:o + 1],
                        rhs=encb[:, o, h2 * 512:(h2 + 1) * 512],
                        start=(o == 0),
                        stop=(o == TT - 1),
                    )
            ctx_sb = sp.tile([1, H], f32, tag="ctx", name=f"ctx{b}")
            nc.vector.tensor_scalar_mul(out=ctx_sb[:], in0=pctx[:], scalar1=rinv[:])
            od = nc.gpsimd.dma_start(out=octx.ap()[b:b + 1, :], in_=ctx_sb[:])
            copy_slots.setdefault((min(b + 1, BLOC - 1), 2 if b < BLOC - 1 else 3), []).append(od)

        # Pin the global DMA order: [consts..w2] -> burst(0,0) -> ... with
        # plain copies only in their designated inter-burst gaps.
        # add_dep_helper(dependent, dependency).
        order = [(b, tb) for b in range(BLOC) for tb in range(NTB)]
        # Head: const copies (w1 SWDGE || w2 ACT-ring) first, then the
        # transpose stream begins.
        for cp in const_lds:
            add_dep_helper(
                tr_groups[order[0]][0].ins, cp.ins, sync=False,
                reason="first transpose burst after const copies",
            )
        prev_tr = None
        for key in order:
            for tr in tr_groups[key]:
                if prev_tr is not None:
                    add_dep_helper(
                        tr.ins, prev_tr.ins, sync=False,
                        reason="transpose stream total order",
                    )
                prev_tr = tr
        for gi, key in enumerate(order):
            grp = tr_groups[key]
            nxt = tr_groups[order[gi + 1]][0] if gi + 1 < len(order) else None
            for cp in copy_slots.get(key, []):
                add_dep_helper(
                    cp.ins, grp[-1].ins, sync=False,
                    reason="plain copy after transpose burst",
                )
                if nxt is not None:
                    add_dep_helper(
                        nxt.ins, cp.ins, sync=False,
                        reason="next transpose burst after gap copies",
                    )

    nc.finalize()
    return nc


def kernel(dec_hidden, all_enc_hiddens, W1, b1, W2, b2, V, bv):
    from concourse.bass_utils import run_bass_kernel_spmd

    if "nc" not in _CACHE:
        _CACHE["nc"] = _build()
    nc = _CACHE["nc"]

    import ml_dtypes
    bf = ml_dtypes.bfloat16
    dec_hidden = np.asarray(dec_hidden, dtype=np.float32)
    all_enc_hiddens = np.ascontiguousarray(np.asarray(all_enc_hiddens, dtype=np.float32).astype(bf))
    W1 = np.ascontiguousarray(np.asarray(W1, dtype=np.float32).astype(bf))
    W2 = np.ascontiguousarray(np.asarray(W2, dtype=np.float32).astype(bf))
    b1 = np.asarray(b1, dtype=np.float32)
    b2 = np.asarray(b2, dtype=np.float32)
    V = np.asarray(V, dtype=np.float32)

    # Host-side layout prep for the small tensors (see _build docstring).
    biasu = np.ascontiguousarray((b1 + b2).reshape(UC, P).T.astype(np.float32))
    vt = np.ascontiguousarray(V[:, 0].reshape(UC, P).T.astype(bf))

    in_maps = []
    for i in range(N_CORES):
        sl = slice(i * BLOC, (i + 1) * BLOC)
        dec_core = dec_hidden[sl]  # [BLOC, H]
        dect = np.ascontiguousarray(
            dec_core.T.reshape(HC, P, BLOC).transpose(1, 0, 2).astype(bf)
        )
        in_maps.append({
            "enc": np.ascontiguousarray(all_enc_hiddens[sl]),
            "w1": W1, "w2": W2, "dect": dect, "biasu": biasu, "vt": vt,
        })

    res = run_bass_kernel_spmd(nc, in_maps, core_ids=list(range(N_CORES)))
    context = np.concatenate([res.results[i]["out_ctx"] for i in range(N_CORES)], axis=0)
    attn = np.concatenate([res.results[i]["out_attn"] for i in range(N_CORES)], axis=0)
    return context.astype(np.float32), attn.reshape(B, T, 1).astype(np.float32)


# revision 67
# speedup vs baseline: 1.0198x; 1.0003x over previous
"""Bahdanau additive attention on 8 Trainium2 NeuronCores.

Reference computation (per full batch B=32, T=2048, H=U=1024):
    q = dec @ W1 + b1                         [B, 1, U]
    k = enc @ W2 + b2                         [B, T, U]
    score = tanh(q + k) @ V + bv              [B, T, 1]
    attn = softmax(score, axis=T)             [B, T, 1]
    context = sum(attn * enc, axis=T)         [B, H]
    returns (context, attn)

Sharding: data-parallel over batch, 4 batches per core; weights replicated.
enc/W1/W2/V are converted to bf16 on the host (total error ~3e-3 rel vs a
2e-2 gate); all matmuls run bf16 with f32 PSUM accumulation.

Per-core dataflow:
    - kT per 512-t block: xbar DMA-transpose straight from DRAM gives
      encT [h,t]; W2 chunks (stationary) x encT -> PSUM [u,t].
    - tanh fused with the +q per-partition bias on ScalarE out of PSUM.
    - score via V.T (stationary) x tanh tile -> PSUM [1,t], pipelined one
      uc-chain behind the k-projection so the PE never waits on ACT.
    - softmax shift-free: e=exp(s); attn = e / sum(e)  (bv cancels).
    - scores transposed [1,T]->[T,1] chunks on TensorE; eT=exp -> lhsT for
      the context matmul against a natural-layout enc copy.

Scheduling notes (the perf-critical part):
    - Tile serializes DMA_TRANSPOSE against in-flight plain-copy DMAs
      (xbar-mode transition guard) and its scheduler interleaves them
      freely, which costs a drain at every mode switch. The transpose
      stream is therefore pinned into a total order with plain copies
      confined to designated inter-burst gaps (add_dep_helper edges).
    - Startup weight loads ride three parallel DMA paths (SWDGE + both
      HWDGE rings); score-phase copies run on DVE to keep the ACT queue
      a pure tanh stream.
"""

import numpy as np

P = 128
B = 32
N_CORES = 8
BLOC = B // N_CORES  # 4 batches per core
T = 2048
H = 1024
U = 1024
HC = H // P  # 8 h-chunks
UC = U // P  # 8 u-chunks
TT = T // P  # 16 t-tiles per batch
TB = 4       # t-tiles per t-block
NTB = TT // TB  # 4 t-blocks
TN = TB * P  # 512

_CACHE = {}


def _build():
    import concourse.mybir as mybir
    import concourse.tile as tile
    from concourse import bacc
    from concourse.masks import make_identity
    from concourse.tile import add_dep_helper

    f32 = mybir.dt.float32
    bf16 = mybir.dt.bfloat16
    AF = mybir.ActivationFunctionType

    nc = bacc.Bacc(None, target_bir_lowering=False)

    # Host pre-arranges small tensors into on-chip layouts:
    #   dect[p, hc, b] = dec[b, hc*128+p] (bf16)
    #   biasu[p, uc]   = b1[uc*128+p] + b2[uc*128+p] (f32)
    #   vt[p, uc]      = V[uc*128+p, 0] (bf16)
    enc = nc.declare_dram_parameter("enc", [BLOC, T, H], bf16, isOutput=False)
    w1 = nc.declare_dram_parameter("w1", [H, U], bf16, isOutput=False)
    w2 = nc.declare_dram_parameter("w2", [H, U], bf16, isOutput=False)
    dect = nc.declare_dram_parameter("dect", [P, HC, BLOC], bf16, isOutput=False)
    biasp = nc.declare_dram_parameter("biasu", [P, UC], f32, isOutput=False)
    vtp = nc.declare_dram_parameter("vt", [P, UC], bf16, isOutput=False)
    octx = nc.declare_dram_parameter("out_ctx", [BLOC, H], f32, isOutput=True)
    oattn = nc.declare_dram_parameter("out_attn", [BLOC, T], f32, isOutput=True)

    from contextlib import ExitStack

    with tile.TileContext(nc) as tc, ExitStack() as ctx:
        consts = ctx.enter_context(tc.tile_pool(name="consts", bufs=1))
        encp = ctx.enter_context(tc.tile_pool(name="encp", bufs=2))
        encTp = ctx.enter_context(tc.tile_pool(name="encTp", bufs=3))
        thp = ctx.enter_context(tc.tile_pool(name="thp", bufs=4))
        sp = ctx.enter_context(tc.tile_pool(name="sp", bufs=2))
        psk = ctx.enter_context(tc.tile_pool(name="psk", bufs=4, space="PSUM"))
        pss = ctx.enter_context(tc.tile_pool(name="pss", bufs=1, space="PSUM"))
        psm = ctx.enter_context(tc.tile_pool(name="psm", bufs=1, space="PSUM"))
        pst = ctx.enter_context(tc.tile_pool(name="pst", bufs=1, space="PSUM"))

        # ---- constants ----
        # ALL plain-copy DMAs ride gpsimd/SWDGE; the sync HWDGE ring carries
        # ONLY the xbar transposes. Rationale: Tile serializes DMA_TRANSPOSE
        # against in-flight HWDGE plain copies (xbar-mode transition guard),
        # but SWDGE copies don't participate — keeping the two streams on
        # disjoint paths lets them pipeline freely.
        const_lds = []
        decT = consts.tile([P, HC, BLOC], bf16)
        const_lds.append(nc.gpsimd.dma_start(out=decT[:], in_=dect.ap()))
        biasu = consts.tile([P, UC], f32)
        const_lds.append(nc.gpsimd.dma_start(out=biasu[:], in_=biasp.ap()))
        vsb = consts.tile([P, UC], bf16)
        const_lds.append(nc.gpsimd.dma_start(out=vsb[:], in_=vtp.ap()))
        # w1 leads the sync HWDGE ring (transposes follow it there).
        w1sb = consts.tile([P, HC, U], bf16)
        const_lds.append(nc.sync.dma_start(
            out=w1sb[:], in_=w1.ap().rearrange("(hc p) u -> p hc u", p=P)))
        # w2 on the ACT HWDGE ring, in parallel with w1 on the SWDGE queue.
        w2sb = consts.tile([P, HC, U], bf16)
        const_lds.append(nc.scalar.dma_start(
            out=w2sb[:], in_=w2.ap().rearrange("(hc p) u -> p hc u", p=P)))
        ident = consts.tile([P, P], f32)
        make_identity(nc, ident[:])

        # qT[u, b] = (dec @ W1)[b, u] + b1[u] + b2[u]
        qT = consts.tile([P, UC, BLOC], f32)
        for uc in range(UC):
            qpsum = psk.tile([P, TN], f32, tag="k", name=f"qpsum{uc}")
            for hc in range(HC):
                nc.tensor.matmul(
                    qpsum[:, :BLOC],
                    lhsT=w1sb[:, hc, uc * P:(uc + 1) * P],
                    rhs=decT[:, hc, :],
                    start=(hc == 0),
                    stop=(hc == HC - 1),
                )
            # On DVE (not ACT) so it can't sit behind a DMA emission on the
            # ACT queue while the PE waits on the qpsum slot.
            nc.vector.tensor_scalar_add(
                out=qT[:, uc, :], in0=qpsum[:, :BLOC], scalar1=biasu[:, uc:uc + 1]
            )

        # ---- main loop over local batches ----
        # Every transpose<->copy mode transition on the DMA path costs a
        # drain (Tile's xbar-transition guard), and the scheduler interleaves
        # them freely. Pin a global order with explicit ordering edges:
        # copies only in designated gaps between transpose bursts.
        tr_groups = {}   # (b, tb) -> [transpose insts]
        copy_slots = {}  # (b, tb) -> [plain-copy insts to place after group]
        encb_tiles = []
        for b in range(BLOC):
            encb = encp.tile([P, TT, H], bf16, tag="enc", name=f"encb{b}")
            encb_tiles.append(encb)
            enc_src = enc.ap()[b].rearrange("(tt p) h -> p tt h", p=P)
            ssb = sp.tile([1, T], f32, tag="s", name=f"s{b}")
            if b == BLOC - 1:
                eTl = sp.tile([P, TT], bf16, tag="eT", name=f"eTl{b}")
            # Natural-layout copy (context matmul input), scheduled into the
            # gap after this batch's tb1 transpose burst.
            ld = nc.gpsimd.dma_start(out=encb[:], in_=enc_src)
            copy_slots.setdefault((b, 1), []).append(ld)
            for tb in range(NTB):
                encT = encTp.tile([P, HC, TB, P], bf16, tag="encT", name=f"encT{b}_{tb}")
                grp = []
                for tt in range(TB):
                    # xbar transpose straight from DRAM: [128t, 1024h] -> [128h, 8hc, 128t]
                    grp.append(nc.sync.dma_start_transpose(
                        encT[:, :, tt, :],
                        enc.ap()[b, (tb * TB + tt) * P:(tb * TB + tt + 1) * P, :],
                    ))
                tr_groups[(b, tb)] = grp
                pscore = pss.tile([1, TN], f32, tag="ps", name=f"pscore{b}_{tb}")
                # k-projection chains, with the V-score matmul pipelined one
                # uc-chain behind so the PE never waits on the tanh drain.
                ths = []
                for uc in range(UC):
                    pk = psk.tile([P, TN], f32, tag="k", name=f"pk{b}_{tb}_{uc}")
                    for hc in range(HC):
                        nc.tensor.matmul(
                            pk[:],
                            lhsT=w2sb[:, hc, uc * P:(uc + 1) * P],
                            rhs=encT[:, hc, :, :],
                            start=(hc == 0),
                            stop=(hc == HC - 1),
                        )
                    th = thp.tile([P, TN], bf16, tag="th", name=f"th{b}_{tb}_{uc}")
                    nc.scalar.activation(
                        out=th[:], in_=pk[:], func=AF.Tanh,
                        bias=qT[:, uc, b:b + 1], scale=1.0,
                    )
                    ths.append(th)
                    if uc >= 1:
                        nc.tensor.matmul(
                            pscore[:],
                            lhsT=vsb[:, uc - 1:uc],
                            rhs=ths[uc - 1][:],
                            start=(uc - 1 == 0),
                            stop=False,
                        )
                nc.tensor.matmul(
                    pscore[:],
                    lhsT=vsb[:, UC - 1:UC],
                    rhs=ths[UC - 1][:],
                    start=False,
                    stop=True,
                )
                # DVE, not ACT: keeps the tanh stream uninterrupted and the
                # score->transpose chain off the busy ACT queue.
                nc.vector.tensor_copy(out=ssb[:, tb * TN:(tb + 1) * TN], in_=pscore[:])
                if b == BLOC - 1:
                    # Last batch: transpose+exp this block's scores right
                    # away — there is no next batch to hide the clump under,
                    # so shrinking the serial tail matters here.
                    pTi = pst.tile([P, TB], f32, tag="pT", name=f"pTi{b}_{tb}")
                    for o4 in range(TB):
                        o = tb * TB + o4
                        nc.tensor.transpose(
                            pTi[:, o4:o4 + 1], ssb[:, o * P:(o + 1) * P], ident[:1, :1])
                    nc.scalar.activation(
                        out=eTl[:, tb * TB:(tb + 1) * TB], in_=pTi[:], func=AF.Exp)

            # softmax (shift-free: scores are O(1) here)
            esb = sp.tile([1, T], f32, tag="e", name=f"e{b}")
            nc.scalar.activation(out=esb[:], in_=ssb[:], func=AF.Exp)
            ssum = sp.tile([1, 1], f32, tag="ssum", name=f"ssum{b}")
            nc.vector.reduce_sum(out=ssum[:], in_=esb[:], axis=mybir.AxisListType.X)
            rinv = sp.tile([1, 1], f32, tag="rinv", name=f"rinv{b}")
            nc.vector.reciprocal(out=rinv[:], in_=ssum[:])
            attn_sb = sp.tile([1, T], f32, tag="attn", name=f"attn{b}")
            nc.vector.tensor_scalar_mul(out=attn_sb[:], in0=esb[:], scalar1=rinv[:])
            od = nc.gpsimd.dma_start(out=oattn.ap()[b:b + 1, :], in_=attn_sb[:])
            # outputs of batch b land in the gap after batch b+1's tb2 burst
            # (data is long ready by then); last batch goes after the end.
            copy_slots.setdefault((min(b + 1, BLOC - 1), 2 if b < BLOC - 1 else 3), []).append(od)

            # scores -> [t, 1] chunks on TensorE, then eT = exp(sT) as bf16
            # (already produced per t-block for the last batch)
            if b == BLOC - 1:
                eT = eTl
            else:
                pT = pst.tile([P, TT], f32, tag="pT", name=f"pT{b}")
                for o in range(TT):
                    nc.tensor.transpose(pT[:, o:o + 1], ssb[:, o * P:(o + 1) * P], ident[:1, :1])
                eT = sp.tile([P, TT], bf16, tag="eT", name=f"eT{b}")
                nc.scalar.activation(out=eT[:], in_=pT[:], func=AF.Exp)

            # context = (eT / Z).T @ enc
            pctx = psm.tile([1, H], f32, tag="pctx", name=f"pctx{b}")
            for o in range(TT):
                for h2 in range(2):
                    nc.tensor.matmul(
                        pctx[:, h2 * 512:(h2 + 1) * 512],
                        lhsT=eT[:, o:o + 1],
                        rhs=encb[:, o, h2 * 512:(h2 + 1) * 512],
                        start=(o == 0),
                        stop=(o == TT - 1),
                    )
            ctx_sb = sp.tile([1, H], f32, tag="ctx", name=f"ctx{b}")
            nc.vector.tensor_scalar_mul(out=ctx_sb[:], in0=pctx[:], scalar1=rinv[:])
            od = nc.gpsimd.dma_start(out=octx.ap()[b:b + 1, :], in_=ctx_sb[:])
            copy_slots.setdefault((min(b + 1, BLOC - 1), 2 if b < BLOC - 1 else 3), []).append(od)

        # Pin the global DMA order: [consts..w2] -> burst(0,0) -> ... with
        # plain copies only in their designated inter-burst gaps.
        # add_dep_helper(dependent, dependency).
        order = [(b, tb) for b in range(BLOC) for tb in range(NTB)]
        # Head: const copies (w1 SWDGE || w2 ACT-ring) first, then the
        # transpose stream begins.
        for cp in const_lds:
            add_dep_helper(
                tr_groups[order[0]][0].ins, cp.ins, sync=False,
                reason="first transpose burst after const copies",
            )
        prev_tr = None
        for key in order:
            for tr in tr_groups[key]:
                if prev_tr is not None:
                    add_dep_helper(
                        tr.ins, prev_tr.ins, sync=False,
                        reason="transpose stream total order",
                    )
                prev_tr = tr
        for gi, key in enumerate(order):
            grp = tr_groups[key]
            nxt = tr_groups[order[gi + 1]][0] if gi + 1 < len(order) else None
            for cp in copy_slots.get(key, []):
                add_dep_helper(
                    cp.ins, grp[-1].ins, sync=False,
                    reason="plain copy after transpose burst",
                )
                if nxt is not None:
                    add_dep_helper(
                        nxt.ins, cp.ins, sync=False,
                        reason="next transpose burst after gap copies",
                    )

    nc.finalize()
    return nc


def kernel(dec_hidden, all_enc_hiddens, W1, b1, W2, b2, V, bv):
    from concourse.bass_utils import run_bass_kernel_spmd

    if "nc" not in _CACHE:
        _CACHE["nc"] = _build()
    nc = _CACHE["nc"]

    import ml_dtypes
    bf = ml_dtypes.bfloat16
    dec_hidden = np.asarray(dec_hidden, dtype=np.float32)
    all_enc_hiddens = np.ascontiguousarray(np.asarray(all_enc_hiddens, dtype=np.float32).astype(bf))
    W1 = np.ascontiguousarray(np.asarray(W1, dtype=np.float32).astype(bf))
    W2 = np.ascontiguousarray(np.asarray(W2, dtype=np.float32).astype(bf))
    b1 = np.asarray(b1, dtype=np.float32)
    b2 = np.asarray(b2, dtype=np.float32)
    V = np.asarray(V, dtype=np.float32)

    # Host-side layout prep for the small tensors (see _build docstring).
    biasu = np.ascontiguousarray((b1 + b2).reshape(UC, P).T.astype(np.float32))
    vt = np.ascontiguousarray(V[:, 0].reshape(UC, P).T.astype(bf))

    in_maps = []
    for i in range(N_CORES):
        sl = slice(i * BLOC, (i + 1) * BLOC)
        dec_core = dec_hidden[sl]  # [BLOC, H]
        dect = np.ascontiguousarray(
            dec_core.T.reshape(HC, P, BLOC).transpose(1, 0, 2).astype(bf)
        )
        in_maps.append({
            "enc": np.ascontiguousarray(all_enc_hiddens[sl]),
            "w1": W1, "w2": W2, "dect": dect, "biasu": biasu, "vt": vt,
        })

    res = run_bass_kernel_spmd(nc, in_maps, core_ids=list(range(N_CORES)))
    context = np.concatenate([res.results[i]["out_ctx"] for i in range(N_CORES)], axis=0)
    attn = np.concatenate([res.results[i]["out_attn"] for i in range(N_CORES)], axis=0)
    return context.astype(np.float32), attn.reshape(B, T, 1).astype(np.float32)


# revision 68
# speedup vs baseline: 1.0218x; 1.0020x over previous
"""Bahdanau additive attention on 8 Trainium2 NeuronCores.

Reference computation (per full batch B=32, T=2048, H=U=1024):
    q = dec @ W1 + b1                         [B, 1, U]
    k = enc @ W2 + b2                         [B, T, U]
    score = tanh(q + k) @ V + bv              [B, T, 1]
    attn = softmax(score, axis=T)             [B, T, 1]
    context = sum(attn * enc, axis=T)         [B, H]
    returns (context, attn)

Sharding: data-parallel over batch, 4 batches per core; weights replicated.
enc/W1/W2/V are converted to bf16 on the host (total error ~3e-3 rel vs a
2e-2 gate); all matmuls run bf16 with f32 PSUM accumulation.

Per-core dataflow:
    - kT per 512-t block: xbar DMA-transpose straight from DRAM gives
      encT [h,t]; W2 chunks (stationary) x encT -> PSUM [u,t].
    - tanh fused with the +q per-partition bias on ScalarE out of PSUM.
    - score via V.T (stationary) x tanh tile -> PSUM [1,t], pipelined one
      uc-chain behind the k-projection so the PE never waits on ACT.
    - softmax shift-free: e=exp(s); attn = e / sum(e)  (bv cancels).
    - scores transposed [1,T]->[T,1] chunks on TensorE; eT=exp -> lhsT for
      the context matmul against a natural-layout enc copy.

Scheduling notes (the perf-critical part):
    - Tile serializes DMA_TRANSPOSE against in-flight plain-copy DMAs
      (xbar-mode transition guard) and its scheduler interleaves them
      freely, which costs a drain at every mode switch. The transpose
      stream is therefore pinned into a total order with plain copies
      confined to designated inter-burst gaps (add_dep_helper edges).
    - Startup weight loads ride three parallel DMA paths (SWDGE + both
      HWDGE rings); score-phase copies run on DVE to keep the ACT queue
      a pure tanh stream.
"""

import numpy as np

P = 128
B = 32
N_CORES = 8
BLOC = B // N_CORES  # 4 batches per core
T = 2048
H = 1024
U = 1024
HC = H // P  # 8 h-chunks
UC = U // P  # 8 u-chunks
TT = T // P  # 16 t-tiles per batch
TB = 4       # t-tiles per t-block
NTB = TT // TB  # 4 t-blocks
TN = TB * P  # 512

_CACHE = {}


def _build():
    import concourse.mybir as mybir
    import concourse.tile as tile
    from concourse import bacc
    from concourse.masks import make_identity
    from concourse.tile import add_dep_helper

    f32 = mybir.dt.float32
    bf16 = mybir.dt.bfloat16
    AF = mybir.ActivationFunctionType

    nc = bacc.Bacc(None, target_bir_lowering=False)

    # Host pre-arranges small tensors into on-chip layouts:
    #   dect[p, hc, b] = dec[b, hc*128+p] (bf16)
    #   biasu[p, uc]   = b1[uc*128+p] + b2[uc*128+p] (f32)
    #   vt[p, uc]      = V[uc*128+p, 0] (bf16)
    enc = nc.declare_dram_parameter("enc", [BLOC, T, H], bf16, isOutput=False)
    w1 = nc.declare_dram_parameter("w1", [H, U], bf16, isOutput=False)
    w2 = nc.declare_dram_parameter("w2", [H, U], bf16, isOutput=False)
    dect = nc.declare_dram_parameter("dect", [P, HC, BLOC], bf16, isOutput=False)
    biasp = nc.declare_dram_parameter("biasu", [P, UC], f32, isOutput=False)
    vtp = nc.declare_dram_parameter("vt", [P, UC], bf16, isOutput=False)
    octx = nc.declare_dram_parameter("out_ctx", [BLOC, H], f32, isOutput=True)
    oattn = nc.declare_dram_parameter("out_attn", [BLOC, T], f32, isOutput=True)

    from contextlib import ExitStack

    with tile.TileContext(nc) as tc, ExitStack() as ctx:
        consts = ctx.enter_context(tc.tile_pool(name="consts", bufs=1))
        encp = ctx.enter_context(tc.tile_pool(name="encp", bufs=2))
        encTp = ctx.enter_context(tc.tile_pool(name="encTp", bufs=3))
        thp = ctx.enter_context(tc.tile_pool(name="thp", bufs=4))
        sp = ctx.enter_context(tc.tile_pool(name="sp", bufs=2))
        psk = ctx.enter_context(tc.tile_pool(name="psk", bufs=3, space="PSUM"))
        pss = ctx.enter_context(tc.tile_pool(name="pss", bufs=2, space="PSUM"))
        psm = ctx.enter_context(tc.tile_pool(name="psm", bufs=1, space="PSUM"))
        pst = ctx.enter_context(tc.tile_pool(name="pst", bufs=1, space="PSUM"))

        # ---- constants ----
        # ALL plain-copy DMAs ride gpsimd/SWDGE; the sync HWDGE ring carries
        # ONLY the xbar transposes. Rationale: Tile serializes DMA_TRANSPOSE
        # against in-flight HWDGE plain copies (xbar-mode transition guard),
        # but SWDGE copies don't participate — keeping the two streams on
        # disjoint paths lets them pipeline freely.
        const_lds = []
        decT = consts.tile([P, HC, BLOC], bf16)
        const_lds.append(nc.gpsimd.dma_start(out=decT[:], in_=dect.ap()))
        biasu = consts.tile([P, UC], f32)
        const_lds.append(nc.gpsimd.dma_start(out=biasu[:], in_=biasp.ap()))
        vsb = consts.tile([P, UC], bf16)
        const_lds.append(nc.gpsimd.dma_start(out=vsb[:], in_=vtp.ap()))
        # w1 leads the sync HWDGE ring (transposes follow it there).
        w1sb = consts.tile([P, HC, U], bf16)
        const_lds.append(nc.sync.dma_start(
            out=w1sb[:], in_=w1.ap().rearrange("(hc p) u -> p hc u", p=P)))
        # w2 on the ACT HWDGE ring, in parallel with w1 on the SWDGE queue.
        w2sb = consts.tile([P, HC, U], bf16)
        const_lds.append(nc.scalar.dma_start(
            out=w2sb[:], in_=w2.ap().rearrange("(hc p) u -> p hc u", p=P)))
        ident = consts.tile([P, P], f32)
        make_identity(nc, ident[:])

        # qT[u, b] = (dec @ W1)[b, u] + b1[u] + b2[u]
        qT = consts.tile([P, UC, BLOC], f32)
        for uc in range(UC):
            qpsum = psk.tile([P, TN], f32, tag="k", name=f"qpsum{uc}")
            for hc in range(HC):
                nc.tensor.matmul(
                    qpsum[:, :BLOC],
                    lhsT=w1sb[:, hc, uc * P:(uc + 1) * P],
                    rhs=decT[:, hc, :],
                    start=(hc == 0),
                    stop=(hc == HC - 1),
                )
            # On DVE (not ACT) so it can't sit behind a DMA emission on the
            # ACT queue while the PE waits on the qpsum slot.
            nc.vector.tensor_scalar_add(
                out=qT[:, uc, :], in0=qpsum[:, :BLOC], scalar1=biasu[:, uc:uc + 1]
            )

        # ---- main loop over local batches ----
        # Every transpose<->copy mode transition on the DMA path costs a
        # drain (Tile's xbar-transition guard), and the scheduler interleaves
        # them freely. Pin a global order with explicit ordering edges:
        # copies only in designated gaps between transpose bursts.
        tr_groups = {}   # (b, tb) -> [transpose insts]
        copy_slots = {}  # (b, tb) -> [plain-copy insts to place after group]
        encb_tiles = []
        for b in range(BLOC):
            encb = encp.tile([P, TT, H], bf16, tag="enc", name=f"encb{b}")
            encb_tiles.append(encb)
            enc_src = enc.ap()[b].rearrange("(tt p) h -> p tt h", p=P)
            ssb = sp.tile([1, T], f32, tag="s", name=f"s{b}")
            if b == BLOC - 1:
                eTl = sp.tile([P, TT], bf16, tag="eT", name=f"eTl{b}")
            # Natural-layout copy (context matmul input), scheduled into the
            # gap after this batch's tb1 transpose burst.
            ld = nc.gpsimd.dma_start(out=encb[:], in_=enc_src)
            copy_slots.setdefault((b, 1), []).append(ld)
            for tb in range(NTB):
                encT = encTp.tile([P, HC, TB, P], bf16, tag="encT", name=f"encT{b}_{tb}")
                grp = []
                for tt in range(TB):
                    # xbar transpose straight from DRAM: [128t, 1024h] -> [128h, 8hc, 128t]
                    grp.append(nc.sync.dma_start_transpose(
                        encT[:, :, tt, :],
                        enc.ap()[b, (tb * TB + tt) * P:(tb * TB + tt + 1) * P, :],
                    ))
                tr_groups[(b, tb)] = grp
                pscore = pss.tile([1, TN], f32, tag="ps", name=f"pscore{b}_{tb}")
                # k-projection chains, with the V-score matmul pipelined one
                # uc-chain behind so the PE never waits on the tanh drain.
                ths = []
                for uc in range(UC):
                    pk = psk.tile([P, TN], f32, tag="k", name=f"pk{b}_{tb}_{uc}")
                    for hc in range(HC):
                        nc.tensor.matmul(
                            pk[:],
                            lhsT=w2sb[:, hc, uc * P:(uc + 1) * P],
                            rhs=encT[:, hc, :, :],
                            start=(hc == 0),
                            stop=(hc == HC - 1),
                        )
                    th = thp.tile([P, TN], bf16, tag="th", name=f"th{b}_{tb}_{uc}")
                    nc.scalar.activation(
                        out=th[:], in_=pk[:], func=AF.Tanh,
                        bias=qT[:, uc, b:b + 1], scale=1.0,
                    )
                    ths.append(th)
                    if uc >= 1:
                        nc.tensor.matmul(
                            pscore[:],
                            lhsT=vsb[:, uc - 1:uc],
                            rhs=ths[uc - 1][:],
                            start=(uc - 1 == 0),
                            stop=False,
                        )
                nc.tensor.matmul(
                    pscore[:],
                    lhsT=vsb[:, UC - 1:UC],
                    rhs=ths[UC - 1][:],
                    start=False,
                    stop=True,
                )
                # DVE, not ACT: keeps the tanh stream uninterrupted and the
                # score->transpose chain off the busy ACT queue.
                nc.vector.tensor_copy(out=ssb[:, tb * TN:(tb + 1) * TN], in_=pscore[:])
                if b == BLOC - 1:
                    # Last batch: transpose+exp this block's scores right
                    # away — there is no next batch to hide the clump under,
                    # so shrinking the serial tail matters here.
                    pTi = pst.tile([P, TB], f32, tag="pT", name=f"pTi{b}_{tb}")
                    for o4 in range(TB):
                        o = tb * TB + o4
                        nc.tensor.transpose(
                            pTi[:, o4:o4 + 1], ssb[:, o * P:(o + 1) * P], ident[:1, :1])
                    nc.scalar.activation(
                        out=eTl[:, tb * TB:(tb + 1) * TB], in_=pTi[:], func=AF.Exp)

            # softmax (shift-free: scores are O(1) here)
            esb = sp.tile([1, T], f32, tag="e", name=f"e{b}")
            nc.scalar.activation(out=esb[:], in_=ssb[:], func=AF.Exp)
            ssum = sp.tile([1, 1], f32, tag="ssum", name=f"ssum{b}")
            nc.vector.reduce_sum(out=ssum[:], in_=esb[:], axis=mybir.AxisListType.X)
            rinv = sp.tile([1, 1], f32, tag="rinv", name=f"rinv{b}")
            nc.vector.reciprocal(out=rinv[:], in_=ssum[:])
            attn_sb = sp.tile([1, T], f32, tag="attn", name=f"attn{b}")
            nc.vector.tensor_scalar_mul(out=attn_sb[:], in0=esb[:], scalar1=rinv[:])
            od = nc.gpsimd.dma_start(out=oattn.ap()[b:b + 1, :], in_=attn_sb[:])
            # outputs of batch b land in the gap after batch b+1's tb2 burst
            # (data is long ready by then); last batch goes after the end.
            copy_slots.setdefault((min(b + 1, BLOC - 1), 2 if b < BLOC - 1 else 3), []).append(od)

            # scores -> [t, 1] chunks on TensorE, then eT = exp(sT) as bf16
            # (already produced per t-block for the last batch)
            if b == BLOC - 1:
                eT = eTl
            else:
                pT = pst.tile([P, TT], f32, tag="pT", name=f"pT{b}")
                for o in range(TT):
                    nc.tensor.transpose(pT[:, o:o + 1], ssb[:, o * P:(o + 1) * P], ident[:1, :1])
                eT = sp.tile([P, TT], bf16, tag="eT", name=f"eT{b}")
                nc.scalar.activation(out=eT[:], in_=pT[:], func=AF.Exp)

            # context = (eT / Z).T @ enc
            pctx = psm.tile([1, H], f32, tag="pctx", name=f"pctx{b}")
            for o in range(TT):
                for h2 in range(2):
                    nc.tensor.matmul(
                        pctx[:, h2 * 512:(h2 + 1) * 512],
                        lhsT=eT[:, o:o + 1],
                        rhs=encb[:, o, h2 * 512:(h2 + 1) * 512],
                        start=(o == 0),
                        stop=(o == TT - 1),
                    )
            ctx_sb = sp.tile([1, H], f32, tag="ctx", name=f"ctx{b}")
            nc.vector.tensor_scalar_mul(out=ctx_sb[:], in0=pctx[:], scalar1=rinv[:])
            od = nc.gpsimd.dma_start(out=octx.ap()[b:b + 1, :], in_=ctx_sb[:])
            copy_slots.setdefault((min(b + 1, BLOC - 1), 2 if b < BLOC - 1 else 3), []).append(od)

        # Pin the global DMA order: [consts..w2] -> burst(0,0) -> ... with
        # plain copies only in their designated inter-burst gaps.
        # add_dep_helper(dependent, dependency).
        order = [(b, tb) for b in range(BLOC) for tb in range(NTB)]
        # Head: const copies (w1 SWDGE || w2 ACT-ring) first, then the
        # transpose stream begins.
        for cp in const_lds:
            add_dep_helper(
                tr_groups[order[0]][0].ins, cp.ins, sync=False,
                reason="first transpose burst after const copies",
            )
        prev_tr = None
        for key in order:
            for tr in tr_groups[key]:
                if prev_tr is not None:
                    add_dep_helper(
                        tr.ins, prev_tr.ins, sync=False,
                        reason="transpose stream total order",
                    )
                prev_tr = tr
        for gi, key in enumerate(order):
            grp = tr_groups[key]
            nxt = tr_groups[order[gi + 1]][0] if gi + 1 < len(order) else None
            for cp in copy_slots.get(key, []):
                add_dep_helper(
                    cp.ins, grp[-1].ins, sync=False,
                    reason="plain copy after transpose burst",
                )
                if nxt is not None:
                    add_dep_helper(
                        nxt.ins, cp.ins, sync=False,
                        reason="next transpose burst after gap copies",
                    )

    nc.finalize()
    return nc


def kernel(dec_hidden, all_enc_hiddens, W1, b1, W2, b2, V, bv):
    from concourse.bass_utils import run_bass_kernel_spmd

    if "nc" not in _CACHE:
        _CACHE["nc"] = _build()
    nc = _CACHE["nc"]

    import ml_dtypes
    bf = ml_dtypes.bfloat16
    dec_hidden = np.asarray(dec_hidden, dtype=np.float32)
    all_enc_hiddens = np.ascontiguousarray(np.asarray(all_enc_hiddens, dtype=np.float32).astype(bf))
    W1 = np.ascontiguousarray(np.asarray(W1, dtype=np.float32).astype(bf))
    W2 = np.ascontiguousarray(np.asarray(W2, dtype=np.float32).astype(bf))
    b1 = np.asarray(b1, dtype=np.float32)
    b2 = np.asarray(b2, dtype=np.float32)
    V = np.asarray(V, dtype=np.float32)

    # Host-side layout prep for the small tensors (see _build docstring).
    biasu = np.ascontiguousarray((b1 + b2).reshape(UC, P).T.astype(np.float32))
    vt = np.ascontiguousarray(V[:, 0].reshape(UC, P).T.astype(bf))

    in_maps = []
    for i in range(N_CORES):
        sl = slice(i * BLOC, (i + 1) * BLOC)
        dec_core = dec_hidden[sl]  # [BLOC, H]
        dect = np.ascontiguousarray(
            dec_core.T.reshape(HC, P, BLOC).transpose(1, 0, 2).astype(bf)
        )
        in_maps.append({
            "enc": np.ascontiguousarray(all_enc_hiddens[sl]),
            "w1": W1, "w2": W2, "dect": dect, "biasu": biasu, "vt": vt,
        })

    res = run_bass_kernel_spmd(nc, in_maps, core_ids=list(range(N_CORES)))
    context = np.concatenate([res.results[i]["out_ctx"] for i in range(N_CORES)], axis=0)
    attn = np.concatenate([res.results[i]["out_attn"] for i in range(N_CORES)], axis=0)
    return context.astype(np.float32), attn.reshape(B, T, 1).astype(np.float32)
